# revision 4
# baseline (speedup 1.0000x reference)
"""MoE layer (top-2 routing, SwiGLU experts) on 8 TRN2 NeuronCores.

Strategy (expert-parallel, matching the sharding hint):
  - Host computes the router (logits -> top-2 -> softmax weights) in f64
    numpy. This is the dispatch decision of the all-to-all; it is ~0.05%
    of the FLOPs. The min gap between the 2nd and 3rd logit is ~1.7e-4,
    so f64 routing agrees with the fp32 reference's selection.
  - Core e receives the tokens routed to expert e (gathered, transposed,
    zero-padded to a static capacity C), expert e's weights
    (pre-transposed on host), and the per-token combine weight.
  - Each core runs the expert FFN: g = x@WgT, u = x@WuT, h = silu(g)*u,
    y = (h@WdT) * combine. Matmul operands are bf16 (same PE rate as
    float32r on TRN2 -- 1 cycle/moving-row -- but half the DMA bytes, and
    SBUF then fits hh + x for the full token capacity, so each weight
    stripe is DMA'd exactly once per invocation), EXCEPT the leading
    ZONE=256 token columns, which run as e4m3 fp8 DoubleRow matmuls at
    ~1.5x the PE rate. Each expert's tokens are sorted ascending by
    combine coefficient so the zone holds the 256 lowest-coefficient
    tokens -- their fp8 error (~6% per token-pair) is damped by the
    small coefficients, keeping end-to-end L2 rel err at ~1.7e-2
    (gate 2e-2). fp8 scales: Wg*32, Wu*8, Wd*64, x unscaled; the silu
    input is rescaled by 1/32 in the ACT op and the 1/512 output scale
    is folded into the zone's combine coefficients on the host. fp8
    operands are declared as uint8 DRAM params (this axon stack cannot
    bind fp8 external inputs) and bitcast to f8 at the matmul APs.
  - Host scatter-adds each expert's scaled output rows into the full
    [T, H] output (the combine of the all-to-all).

Kernel layout per core (C = token capacity, chunks of <=512 tokens):
  pass1 (per i-tile, per chunk): g/u accumulate over 8 h-tiles in PSUM,
    silu on ACT -> hh bf16 in SBUF [i-part, token-free], in-place
    multiply by u on DVE. x arrives as 3 chunk-sized DMAs so the first
    chunk's matmuls start ~3us earlier than a monolithic load allows.
  pass2 (per 128-token tile): y accumulates over 22 i-tiles with
    stationary hh tiles and moving resident-WdT rows -> PSUM
    [c-part, 1024], then ACT copy with per-partition combine scale ->
    SBUF -> DRAM, token-major [C, H].
Measured (robust slope protocol, R=1000 vs 3000): ~234 us per
invocation on-device vs ~310 us for the float32r 2-group predecessor
(~270 us for all-bf16 without the fp8 zone).
End-to-end L2 rel err vs fp32 reference ~1.73e-2 (gate 2e-2).
"""

import sys

if "/opt/trn_rl_repo" not in sys.path:
    sys.path.insert(0, "/opt/trn_rl_repo")

import numpy as np
import ml_dtypes

B, S, H, I, E = 2, 2048, 1024, 2816, 8
T = B * S
HT = H // 128   # 8 h-tiles
IT = I // 128   # 22 i-tiles
TOP_K = 2

_PROG_CACHE = {}


def _split_waits(nc):
    """This walrus build rejects >1 sync wait per instruction; move extra
    waits onto standalone event-sem instructions on the issuing engine.
    For HWDGE DMAs the enqueue happens at engine-execution time, so a
    preceding engine-stream wait still gates the transfer."""
    import concourse.mybir as mybir

    for f in nc.m.functions:
        for blk in f.blocks:
            out = []
            for inst in blk.instructions:
                si = inst.sync_info
                if si is None or len(si.on_wait) <= 1:
                    out.append(inst)
                    continue
                waits = list(si.on_wait)
                for k, w in enumerate(waits[:-1]):
                    ev = mybir.InstEventSemaphore(name=f"{inst.name}_ws{k}")
                    ev.engine = inst.engine
                    ev.sync_info = mybir.SyncInfo(on_wait=[w], on_update=[])
                    out.append(ev)
                while len(si.on_wait) > 1:
                    si.on_wait.pop(0)
                out.append(inst)
            blk.instructions = out


CHUNK_MODE = "mixed"   # "mixed": 512s + tail; "384": all-384 chunks
ZONE = 256             # leading token columns computed in fp8 (DoubleRow)
WBUFS = 5              # wg/wu stripe prefetch depth
GROUP_CAP = 1152       # bf16: full C fits one group (weights stream once)
OUTBUFS = 2            # out staging depth (double-buffered: the final ACT
                       # copies no longer serialize against out-DMA drains)
PASS_FILTER = None     # None | "p1" | "p2"  (diagnostics only)
XT_SPLIT = 3           # xt DMA segments (first chunk lands first)
OUTQ = "sp"            # engine queue for output DMAs: "sp" | "act"


def _chunks_of(C):
    """Split C (multiple of 128) into matmul-N chunks, each a multiple of
    128 with 256 <= cn <= 512. The leading ZONE chunk is the fp8 zone."""
    out = [(0, ZONE)]
    c0 = ZONE
    rem = C - ZONE
    if CHUNK_MODE == "384" and C % 384 == 0:
        while rem > 0:
            out.append((c0, 384))
            c0 += 384
            rem -= 384
        return out
    while rem > 0:
        if rem > 512 and rem < 768:
            cn = rem - 256 if rem - 256 <= 512 else 384
        else:
            cn = min(512, rem)
        out.append((c0, cn))
        c0 += cn
        rem -= cn
    return out


def _build_program(C, repeat=1, bench=False):
    import concourse.bass as bass
    import concourse.mybir as mybir
    from concourse.tile import TileContext

    dt = mybir.dt
    f32 = dt.float32
    bf16 = dt.bfloat16
    u8 = dt.uint8
    f8 = dt.float8e4
    DR = mybir.MatmulPerfMode.DoubleRow
    Silu = mybir.ActivationFunctionType.Silu
    CT = C // 128
    chunks = _chunks_of(C)

    nc = bass.Bass()
    if bench:
        # timing-only build: big tensors live in internal DRAM (no host
        # transfer); only a tiny dummy output is external
        xT = nc.dram_tensor("xT", [H, C], bf16)
        wg = nc.dram_tensor("wg", [IT, 128, H], bf16)
        wu = nc.dram_tensor("wu", [IT, 128, H], bf16)
        wd = nc.dram_tensor("wd", [I, H], bf16)
        xT8 = nc.dram_tensor("xT8", [H, ZONE], u8)
        wgu8 = nc.dram_tensor("wgu8", [IT, 128, 2048], u8)
        wd8 = nc.dram_tensor("wd8", [I, H], u8)
        ce = nc.dram_tensor("ce", [128, CT], f32)
        y = nc.dram_tensor("y", [C, H], f32)
        dummy = nc.declare_dram_parameter("bench_out", [128, 4], f32, isOutput=True)
    else:
        xT = nc.declare_dram_parameter("xT", [H, C], bf16, isOutput=False)
        wg = nc.declare_dram_parameter("wg", [IT, 128, H], bf16, isOutput=False)
        wu = nc.declare_dram_parameter("wu", [IT, 128, H], bf16, isOutput=False)
        wd = nc.declare_dram_parameter("wd", [I, H], bf16, isOutput=False)
        xT8 = nc.declare_dram_parameter("xT8", [H, ZONE], u8, isOutput=False)
        wgu8 = nc.declare_dram_parameter("wgu8", [IT, 128, 2048], u8, isOutput=False)
        wd8 = nc.declare_dram_parameter("wd8", [I, H], u8, isOutput=False)
        ce = nc.declare_dram_parameter("ce", [128, CT], f32, isOutput=False)
        y = nc.declare_dram_parameter("y", [C, H], f32, isOutput=True)

    wd_r = wd.rearrange("(it p) hd -> p it hd", p=128)
    wd8_r = wd8.rearrange("(it p) hd -> p it hd", p=128)
    xT_r = xT.rearrange("(ht p) c -> p ht c", p=128)
    xT8_r = xT8.rearrange("(ht p) c -> p ht c", p=128)

    with TileContext(nc) as tc:
        with (
            tc.tile_pool(name="resident", bufs=1) as resident,
            tc.tile_pool(name="wstripe", bufs=WBUFS) as wstripe,
            tc.tile_pool(name="xtp", bufs=1) as xtpool,
            tc.tile_pool(name="hh", bufs=1) as hhpool,
            tc.tile_pool(name="outp", bufs=OUTBUFS) as outp,
            tc.tile_pool(name="ps1", bufs=2, space="PSUM") as ps1,
            tc.tile_pool(name="ps2", bufs=2, space="PSUM") as ps2,
        ):
            if bench:
                # zero-fill internal tensors so timing data is clean fp
                zt = outp.tile([128, H], f32, tag="out")
                nc.vector.memset(zt[:, :], 0.0)
                ztr = zt[:, :].bitcast(bf16)

                def zfill(t, rows, cols):
                    for r in range(0, rows, 128):
                        for c in range(0, cols, 2 * H):
                            w = min(2 * H, cols - c)
                            nc.sync.dma_start(
                                out=t[r:r + 128, c:c + w], in_=ztr[:, :w]
                            )

                for i in range(IT):
                    nc.sync.dma_start(out=wg[i, :, :], in_=ztr[:, :H])
                    nc.sync.dma_start(out=wu[i, :, :], in_=ztr[:, :H])
                zfill(wd, I, H)
                zfill(xT, H, C)
                ztu = zt[:, :].bitcast(u8)  # [128, 4096] zero bytes
                for i in range(IT):
                    nc.sync.dma_start(out=wgu8[i, :, :], in_=ztu[:, :2048])
                for r in range(0, I, 128):
                    nc.sync.dma_start(out=wd8[r:r + 128, :], in_=ztu[:, :H])
                for r in range(0, H, 128):
                    nc.sync.dma_start(out=xT8[r:r + 128, :], in_=ztu[:, :ZONE])
                nc.sync.dma_start(out=ce[:, :], in_=zt[:, :CT])

            # Resident tensors
            ce_sb = resident.tile([128, CT], f32)
            nc.sync.dma_start(out=ce_sb[:, :], in_=ce[:, :])
            wd_sb = resident.tile([128, IT, H], bf16)
            for i in range(IT):
                nc.sync.dma_start(out=wd_sb[:, i, :], in_=wd_r[:, i, :])
            wd8_sb = resident.tile([128, IT, H], u8)
            for i in range(IT):
                nc.sync.dma_start(out=wd8_sb[:, i, :], in_=wd8_r[:, i, :])

            # group chunks so weight stripes stream once per group while
            # hh (sized to max group width) + resident wd fit in SBUF
            groups = []
            for c0, cn in chunks:
                if groups and sum(c[1] for c in groups[-1]) + cn <= GROUP_CAP:
                    groups[-1].append((c0, cn))
                else:
                    groups.append([(c0, cn)])
            hh_w = max(sum(c[1] for c in grp) for grp in groups)

            def body():
                for grp in groups:
                    g0 = grp[0][0]
                    gw = sum(c[1] for c in grp)
                    hh = hhpool.tile([128, IT, hh_w], bf16, tag="hh")
                    hh8 = hhpool.tile([128, IT, ZONE], f8, tag="hh8")
                    xt_sb = xtpool.tile([128, HT, hh_w], bf16, tag="xt")
                    xt8_sb = xtpool.tile([128, HT, ZONE], u8, tag="xt8")
                    # fp8 zone x first (smallest; zone matmuls start earliest),
                    # then the bf16 chunks
                    nc.sync.dma_start(out=xt8_sb[:, :, :], in_=xT8_r[:, :, :])
                    for c0, cn in grp[1:]:
                        nc.sync.dma_start(
                            out=xt_sb[:, :, c0 - g0:c0 - g0 + cn],
                            in_=xT_r[:, :, c0:c0 + cn],
                        )
                    # ---- pass 1: hh = silu(x@WgT) * (x@WuT) ----
                    for i in range(IT) if PASS_FILTER != "p2" else ():
                        w8 = wstripe.tile([128, 2, 4, 2, 128], u8, tag="w8")
                        nc.sync.dma_start(
                            out=w8[:, :, :, :, :].rearrange(
                                "p a j r m -> p (a j r m)"),
                            in_=wgu8[i, :, :],
                        )
                        wgt = wstripe.tile([128, HT, 128], bf16, tag="wg")
                        wut = wstripe.tile([128, HT, 128], bf16, tag="wu")
                        nc.sync.dma_start(
                            out=wgt[:, :, :].rearrange("p ht c -> p (ht c)"),
                            in_=wg[i, :, :],
                        )
                        nc.sync.dma_start(
                            out=wut[:, :, :].rearrange("p ht c -> p (ht c)"),
                            in_=wu[i, :, :],
                        )
                        # fp8 zone chunk: DoubleRow pairs over 4 h-tile pairs
                        g8_ps = ps1.tile([128, 512], f32, tag="g")
                        u8_ps = ps1.tile([128, 512], f32, tag="u")
                        for j in range(4):
                            nc.tensor.matmul(
                                g8_ps[:, :ZONE],
                                w8[:, 0, j, :, :].bitcast(f8),
                                xt8_sb[:, 2 * j:2 * j + 2, :].bitcast(f8),
                                start=(j == 0),
                                stop=(j == 3),
                                perf_mode=DR,
                            )
                        for j in range(4):
                            nc.tensor.matmul(
                                u8_ps[:, :ZONE],
                                w8[:, 1, j, :, :].bitcast(f8),
                                xt8_sb[:, 2 * j:2 * j + 2, :].bitcast(f8),
                                start=(j == 0),
                                stop=(j == 3),
                                perf_mode=DR,
                            )
                        h8slice = hh8[:, i, :]
                        nc.scalar.activation(h8slice, g8_ps[:, :ZONE], Silu,
                                             scale=1.0 / 32.0)
                        nc.vector.tensor_mul(h8slice, h8slice, u8_ps[:, :ZONE])
                        for c0, cn in grp[1:]:
                            g_ps = ps1.tile([128, 512], f32, tag="g")
                            u_ps = ps1.tile([128, 512], f32, tag="u")
                            for h in range(HT):
                                nc.tensor.matmul(
                                    g_ps[:, :cn],
                                    wgt[:, h, :],
                                    xt_sb[:, h, c0 - g0:c0 - g0 + cn],
                                    start=(h == 0),
                                    stop=(h == HT - 1),
                                )
                                nc.tensor.matmul(
                                    u_ps[:, :cn],
                                    wut[:, h, :],
                                    xt_sb[:, h, c0 - g0:c0 - g0 + cn],
                                    start=(h == 0),
                                    stop=(h == HT - 1),
                                )
                            hslice = hh[:, i, c0 - g0:c0 - g0 + cn]
                            nc.scalar.activation(hslice, g_ps[:, :cn], Silu)
                            nc.vector.tensor_mul(hslice, hslice, u_ps[:, :cn])
                    # ---- pass 2: y = (hh @ WdT) * combine ----
                    for c0, cn in grp if PASS_FILTER != "p1" else ():
                        for ci in range(cn // 128):
                            y_ps = ps2.tile([128, H], f32, tag="y")
                            cs = c0 + ci * 128
                            hs = c0 - g0 + ci * 128
                            if c0 == 0:
                                # fp8 zone tile: DoubleRow pairs over i-tiles
                                for ip in range(IT // 2):
                                    for nh in range(2):
                                        nc.tensor.matmul(
                                            y_ps[:, nh * 512:(nh + 1) * 512],
                                            hh8[:, 2 * ip:2 * ip + 2,
                                                hs:hs + 128],
                                            wd8_sb[:, 2 * ip:2 * ip + 2,
                                                   nh * 512:(nh + 1) * 512
                                                   ].bitcast(f8),
                                            start=(ip == 0),
                                            stop=(ip == IT // 2 - 1),
                                            perf_mode=DR,
                                        )
                            else:
                                for i in range(IT):
                                    for nh in range(2):
                                        nc.tensor.matmul(
                                            y_ps[:, nh * 512:(nh + 1) * 512],
                                            hh[:, i, hs:hs + 128],
                                            wd_sb[:, i,
                                                  nh * 512:(nh + 1) * 512],
                                            start=(i == 0),
                                            stop=(i == IT - 1),
                                        )
                            out_sb = outp.tile([128, H], f32, tag="out")
                            nc.scalar.activation(
                                out_sb[:, :],
                                y_ps[:, :],
                                mybir.ActivationFunctionType.Copy,
                                scale=ce_sb[:, cs // 128:cs // 128 + 1],
                            )
                            (nc.scalar if OUTQ == "act" else nc.sync).dma_start(
                                out=y[cs:cs + 128, :], in_=out_sb[:, :]
                            )

            if repeat == 1:
                body()
            else:
                with tc.For_i(0, repeat, 1):
                    body()

            if bench:
                nc.sync.dma_start(out=dummy[:, :], in_=ce_sb[:, :4])

    _split_waits(nc)
    return nc


def _route(xf, router_w):
    """Host-side router: replicate reference's top-2 + softmax in f64."""
    logits = xf.astype(np.float64) @ router_w.astype(np.float64).T  # [T, E]
    # stable argsort of negated logits == top_k tie-break (lower idx first)
    order = np.argsort(-logits, axis=1, kind="stable")[:, :TOP_K]  # [T, 2]
    top_vals = np.take_along_axis(logits, order, axis=1)
    ex = np.exp(top_vals - top_vals[:, :1])
    top_w = ex / ex.sum(axis=1, keepdims=True)  # [T, 2]
    return order.astype(np.int64), top_w


def kernel(x, router_w, Wg, Wu, Wd):
    from concourse.bass_utils import run_bass_kernel_spmd

    in_dtype = x.dtype
    xf = np.ascontiguousarray(x.reshape(T, H), dtype=np.float32)
    top_idx, top_w = _route(xf, np.asarray(router_w, dtype=np.float32))

    # per-expert token lists, sorted ascending by combine coefficient so
    # the ZONE lowest-coefficient tokens land in the fp8 zone columns
    ids = []
    wts = []
    for e in range(E):
        sel = np.nonzero(top_idx == e)
        w_e = top_w[sel[0], sel[1]].astype(np.float32)
        o = np.argsort(w_e, kind="stable")
        ids.append(sel[0][o])
        wts.append(w_e[o])
    counts = np.array([len(a) for a in ids])
    C = max(512, int(np.ceil(counts.max() / 128)) * 128)

    def q8(a, scale):
        E4 = ml_dtypes.float8_e4m3fn
        return np.clip(np.asarray(a, np.float32) * scale, -240, 240).astype(E4)

    bf = ml_dtypes.bfloat16

    def stripe_pack(w):
        # [I, H] -> [IT, 128p(h%128), HT*128(i-col)] contiguous stripes
        a = np.asarray(w, np.float32).reshape(IT, 128, HT, 128)
        return np.ascontiguousarray(
            a.transpose(0, 3, 2, 1).reshape(IT, 128, H).astype(bf)
        )

    def pack8(w8):
        # [I, H] fp8 -> [IT, 128p(h%128), (j, pair, m)] DoubleRow stripes
        a = w8.reshape(IT, 128, 8, 128)          # [it, m, k8, p]
        a = a.transpose(0, 3, 2, 1)              # [it, p, k8, m]
        return a.reshape(IT, 128, 1024)

    in_maps = []
    for e in range(E):
        n_e = counts[e]
        xT_e = np.zeros((H, C), dtype=bf)
        xT_e[:, :n_e] = xf[ids[e]].T.astype(bf)
        ce_col = np.zeros(C, dtype=np.float32)
        ce_col[:n_e] = wts[e]
        ce_col[:ZONE] /= 512.0   # fp8 zone: y_psum = 512*y
        ce_e = np.ascontiguousarray(ce_col.reshape(C // 128, 128).T)
        x8_cols = q8(xf[ids[e][:ZONE]].T, 1.0)
        if n_e < ZONE:
            pad = np.zeros((H, ZONE), dtype=x8_cols.dtype)
            pad[:, :n_e] = x8_cols
            x8_cols = pad
        wgu8_e = np.concatenate(
            [pack8(q8(Wg[e], 32.0)), pack8(q8(Wu[e], 8.0))], axis=2
        )
        in_maps.append(
            {
                "xT": xT_e,
                "wg": stripe_pack(Wg[e]),
                "wu": stripe_pack(Wu[e]),
                "wd": np.ascontiguousarray(np.asarray(Wd[e], np.float32).T.astype(bf)),
                "xT8": np.ascontiguousarray(x8_cols).view(np.uint8),
                "wgu8": np.ascontiguousarray(wgu8_e).view(np.uint8),
                "wd8": np.ascontiguousarray(q8(Wd[e], 64.0).T).view(np.uint8),
                "ce": ce_e,
            }
        )

    nc = _PROG_CACHE.get(C)
    if nc is None:
        nc = _build_program(C)
        _PROG_CACHE[C] = nc

    res = run_bass_kernel_spmd(nc, in_maps, list(range(E)))

    out = np.zeros((T, H), dtype=np.float32)
    for e in range(E):
        n_e = counts[e]
        np.add.at(out, ids[e], res.results[e]["y"][:n_e])
    return out.reshape(B, S, H).astype(in_dtype, copy=False)

